# revision 1
# baseline (speedup 1.0000x reference)
"""BitLinear kernel for Trainium2, tensor-parallel over 8 NeuronCores.

Reference computation:
    w_q = sign(weight) * mean(|weight|)      # weight [DOUT, DIN]
    out = x @ w_q.T + bias                   # x [B, S, DIN] -> out [B, S, DOUT]

Strategy (v2, fp8 DoubleRow):
  - The PE runs fp8e4m3 matmuls with perf_mode=DoubleRow at 2x bf16
    throughput (measured 216ns per [K=256]x[M=128]x[N=512] mm, exactly
    2x the bf16 rate).  sign(w) is exact in fp8; the only quantization
    error is on x.
  - x is split hi/lo on the host: hi = e4m3(x) over the full K=4096,
    lo = e4m3(x - hi) over the first KLO rows of K.  Error budget:
    sqrt((1-f)*0.0265^2) with f = KLO/K; LO_PAIRS=9 -> f=0.5625 ->
    predicted l2 rel err ~0.0175 (gate 2e-2).  Matmul cost is
    (1+f)/2 = 0.78 of the bf16 roofline.
  - w is cast to bf16 on host (halves the critical w read), sign'd to
    fp8 on device; |w| partial sums reduce on DVE as chunks stream in.
  - The global scale = mean|w| is produced ON DEVICE inside the same
    launch via a tiny cross-core AllReduce of the per-core partial
    sums (fold_scale=True), eliminating the separate launch A.
    Fallback (BITLINEAR_FOLD=0): two launches like v1.
  - Drains are split: ACT copies PSUM->SBUF (frees the bank with no
    dependence on the scale), DVE applies out = raw*scale + bias once
    the AllReduce lands, so the PE never stalls on the scale path.
"""

import os
import sys

for _p in ("/opt/trn_rl_repo",):
    if _p not in sys.path:
        sys.path.insert(0, _p)

from contextlib import ExitStack

import numpy as np
import ml_dtypes

import concourse.bass as bass
import concourse.tile as tile
from concourse import mybir
from concourse.bass_utils import run_bass_kernel_spmd

# ----------------------------------------------------------------------------
# Workaround for a walrus codegen limitation in this container: instructions
# can only encode ONE sync wait; insert single-wait NOPs for the rest.
# ----------------------------------------------------------------------------


def _mint_nop(nc, engine):
    inst = nc.engines[engine].nop(nofuse=True, hint="wsplit").ins
    bb = nc.cur_bb.bb
    lst = bb.instructions
    assert lst[-1].name == inst.name
    lst.pop()
    bb.instructions = lst
    return inst


def _split_multi_waits(nc):
    for fn in nc.m.functions:
        for bb in fn.blocks:
            insts = bb.instructions
            if not any(
                i.sync_info and i.sync_info.on_wait and len(i.sync_info.on_wait) > 1
                for i in insts
            ):
                continue
            new = []
            for inst in insts:
                si = inst.sync_info
                if si and si.on_wait and len(si.on_wait) > 1:
                    waits = list(si.on_wait)
                    for w in waits[:-1]:
                        nop = _mint_nop(nc, inst.engine)
                        nop.sync_info = mybir.SyncInfo(on_wait=[w], on_update=[])
                        new.append(nop)
                    si.on_wait = [waits[-1]]
                new.append(inst)
            bb.instructions = new


# ----------------------------------------------------------------------------
# Problem constants (hardcoded per contract)
# ----------------------------------------------------------------------------

B, S, DIN, DOUT = 2, 4096, 4096, 11008
N_CORES = 8
M = B * S
DOUT_SH = DOUT // N_CORES  # 1376
P = 128
KO = DIN // P  # 32 k-subtiles
KP = KO // 2  # 16 DoubleRow pairs
MT = M // P  # 64 row tiles
LO_PAIRS = int(os.environ.get("BITLINEAR_LO_PAIRS", "8"))
KLO = LO_PAIRS * 256
F32 = mybir.dt.float32
BF16 = mybir.dt.bfloat16
FP8 = mybir.dt.float8e4
NPF8 = ml_dtypes.float8_e4m3
NPBF16 = ml_dtypes.bfloat16
DR = mybir.MatmulPerfMode.DoubleRow

X_W = 512
SUB = X_W // P
N_STEP = 512


def _n_slices(total: int, step: int):
    out = []
    o = 0
    while o < total:
        out.append((o, min(step, total - o)))
        o += step
    return out


# ----------------------------------------------------------------------------
# Main kernel
# ----------------------------------------------------------------------------


def build_main_kernel(fold_scale: bool = True) -> bass.Bass:
    nc = bass.Bass(
        "TRN2",
        target_bir_lowering=False,
        debug=False,
        num_devices=N_CORES if fold_scale else None,
    )
    xhi = nc.dram_tensor("xhi", [DIN, M], FP8, kind="ExternalInput").ap()
    xlo = nc.dram_tensor("xlo", [KLO, M], FP8, kind="ExternalInput").ap()
    wt = nc.dram_tensor("wt", [DIN, DOUT_SH], BF16, kind="ExternalInput").ap()
    bias = nc.dram_tensor("bias", [1, DOUT_SH], F32, kind="ExternalInput").ap()
    if not fold_scale:
        scale_in = nc.dram_tensor("scale", [1, 1], F32, kind="ExternalInput").ap()
    out = nc.dram_tensor("out", [M, DOUT_SH], F32, kind="ExternalOutput").ap()

    xhi4 = xhi.rearrange("(kp i p) m -> p kp i m", i=2, p=P)  # [128,16,2,M]
    xlo4 = xlo.rearrange("(kp i p) m -> p kp i m", i=2, p=P)  # [128,9,2,M]
    wt3 = wt.rearrange("(ko p) n -> p ko n", p=P)  # [128,32,DOUT_SH]
    out3 = out.rearrange("(mt p) n -> p mt n", p=P)  # [128,64,DOUT_SH]

    nsl = _n_slices(DOUT_SH, N_STEP)
    WKB = 2  # w subtiles per chunk (= one DoubleRow pair per chunk)
    NCH = KO // WKB  # 16 chunks

    with tile.TileContext(nc) as tc, ExitStack() as ctx:
        const = ctx.enter_context(tc.tile_pool(name="const", bufs=1))
        wload = ctx.enter_context(tc.tile_pool(name="wload", bufs=5))
        xhp = ctx.enter_context(tc.tile_pool(name="xhp", bufs=2))
        xlp = ctx.enter_context(tc.tile_pool(name="xlp", bufs=2))
        outp = ctx.enter_context(tc.tile_pool(name="outp", bufs=8))
        outp2 = ctx.enter_context(tc.tile_pool(name="outp2", bufs=5))
        psum = ctx.enter_context(tc.tile_pool(name="psum", bufs=8, space="PSUM"))

        # prefetch x group 0 ahead of everything else on the pool queue
        # (group 1, the bias chain and the AR trigger follow; only g0 is
        # needed in the first ~60us, and less early HBM traffic gets the
        # first w chunk - which gates the signs - in sooner)
        xg = {}
        xh0 = xhp.tile([P, KP, 2, X_W], FP8, name="xh")
        nc.gpsimd.dma_start(xh0[:], xhi4[:, :, :, 0:X_W])
        xl0 = xlp.tile([P, LO_PAIRS, 2, X_W], FP8, name="xl")
        nc.gpsimd.dma_start(xl0[:], xlo4[:, :, :, 0:X_W])
        xg[0] = (xh0, xl0)

        # --- bias broadcast on the pool queue (latency-chained doublings
        # would delay the w stream on sync or the signs on ACT; on pool
        # they land by ~25us, well before the first scale+bias STT) ---
        b_rep = const.tile([P, DOUT_SH], F32)
        nc.gpsimd.dma_start(b_rep[0:1, :], bias[:])
        n = 1
        while n < P:
            nc.gpsimd.dma_start(b_rep[n : 2 * n, :], b_rep[0:n, :])
            n *= 2

        # x group 1: after the bias chain, still ahead of the AR trigger
        xh1 = xhp.tile([P, KP, 2, X_W], FP8, name="xh")
        nc.gpsimd.dma_start(xh1[:], xhi4[:, :, :, X_W : 2 * X_W])
        xl1 = xlp.tile([P, LO_PAIRS, 2, X_W], FP8, name="xl")
        nc.gpsimd.dma_start(xl1[:], xlo4[:, :, :, X_W : 2 * X_W])
        xg[1] = (xh1, xl1)

        # --- w stream (sync ring): bf16 chunks -> sign to fp8 + |w| reduce ---

        wq = [
            const.tile([P, 2, DOUT_SH], FP8, tag=f"wq{kp}", name=f"wq{kp}")
            for kp in range(KP)
        ]
        wsum = const.tile([P, NCH], F32)
        for j in range(NCH):
            wtile = wload.tile([P, WKB, DOUT_SH], BF16, name="wtile")
            nc.sync.dma_start(wtile[:], wt3[:, j * WKB : (j + 1) * WKB])
            nc.vector.tensor_reduce(
                wsum[:, j : j + 1],
                wtile[:],
                axis=mybir.AxisListType.XY,
                op=mybir.AluOpType.add,
                apply_absolute_value=True,
            )
            # one ACT op per wq pair (2 contiguous subtiles) halves the
            # per-op overhead on the sign stream
            for t in range(0, WKB, 2):
                kp = (j * WKB + t) // 2
                nc.scalar.sign(wq[kp][:], wtile[:, t : t + 2])

        # --- scale chain ---
        sc_rep = const.tile([P, 1], F32)
        cc_out = None
        if fold_scale:
            # cross-partition collapse without the PE: [128,1] -> DRAM ->
            # re-read as [1,128] -> free-axis reduce
            tot = const.tile([P, 1], F32)
            nc.vector.tensor_reduce(
                tot[:], wsum[:], axis=mybir.AxisListType.X, op=mybir.AluOpType.add
            )
            red_d = nc.dram_tensor("red_bounce", [P, 1], F32).ap()
            nc.sync.dma_start(red_d, tot[:])
            rowt = const.tile([1, P], F32)
            nc.sync.dma_start(rowt[:], red_d.rearrange("p o -> o p"))
            tot1 = const.tile([1, 1], F32)
            nc.vector.tensor_reduce(
                tot1[:], rowt[:], axis=mybir.AxisListType.X, op=mybir.AluOpType.add
            )
            stage = const.tile([P, P], F32)
            nc.vector.memset(stage[:], 0.0)
            nc.vector.tensor_scalar_mul(
                stage[0:1, 0:1], tot1[:], 1.0 / float(DOUT * DIN)
            )
            cc_in = nc.dram_tensor("cc_in", [P, P], F32).ap()
            cc_out = nc.dram_tensor("cc_out", [P, P], F32).ap()
            nc.sync.dma_start(cc_in, stage[:])
            # gpsimd owns collective_compute; x groups 0/1 are already
            # queued ahead of it, and group 2 isn't consumed until well
            # after the AR's input wait clears (~40us).  Readback +
            # broadcast go on the sync ring: behind them are only
            # out-DMAs, which need the scale anyway.
            nc.gpsimd.collective_compute(
                "AllReduce",
                mybir.AluOpType.add,
                replica_groups=[list(range(N_CORES))],
                ins=[cc_in.opt()],
                outs=[cc_out.opt()],
            )
            nc.sync.dma_start(sc_rep[0:1, :], cc_out[0:1, 0:1])
        else:
            nc.sync.dma_start(sc_rep[0:1, :], scale_in[:])
        n = 1
        while n < P:
            nc.sync.dma_start(sc_rep[n : 2 * n, :], sc_rep[0:n, :])
            n *= 2

        # --- main loop over a flat tile list ---
        # tiles[idx] = (mt, n0, nw); mt = idx // len(nsl).
        # The first NPRE tiles run "k-progressive": one wave per wq pair
        # across all NPRE open PSUM banks, so the PE tracks the w stream
        # instead of idling until all pairs are signed.
        NPRE = 8
        n_acc = KP + LO_PAIRS
        NT = len(nsl)
        tiles = [(mt, *nsl[i]) for mt in range(MT) for i in range(NT)]

        def get_x(mt):
            g = mt // SUB
            if g not in xg:
                xh = xhp.tile([P, KP, 2, X_W], FP8, name="xh")
                nc.gpsimd.dma_start(xh[:], xhi4[:, :, :, g * X_W : (g + 1) * X_W])
                xl = xlp.tile([P, LO_PAIRS, 2, X_W], FP8, name="xl")
                nc.gpsimd.dma_start(xl[:], xlo4[:, :, :, g * X_W : (g + 1) * X_W])
                xg[g] = (xh, xl)
            return xg[g], mt % SUB

        ots = {}

        # From this mt on (t >= ~360us), the scale is provably ready even
        # under worst-case AllReduce latency (~190us observed throttled),
        # so drains go PSUM -> STT directly on DVE, skipping the ACT copy
        # (saves ~160us of ACT busy + two extra SBUF passes per tile).
        DIRECT_MT = 24

        def get_ot(mt):
            if mt not in ots:
                ot = None
                if mt < DIRECT_MT:
                    ot = outp.tile([P, DOUT_SH], F32, name="ot")
                ots[mt] = (ot, outp2.tile([P, DOUT_SH], F32, name="ot2"), [0])
            return ots[mt]

        def mm(pt, mt, kp, lo, n0, nw, start, stop):
            (xh, xl), s = get_x(mt)
            xs = xl if lo else xh
            nc.tensor.matmul(
                pt[:, :nw],
                xs[:, kp, :, s * P : (s + 1) * P],
                wq[kp][:, :, n0 : n0 + nw],
                start=start,
                stop=stop,
                perf_mode=DR,
            )

        def drain(pt, mt, n0, nw):
            ot, ot2, cnt = get_ot(mt)
            if ot is not None:
                # copy-drain on ACT (no scale dependence; frees the bank)
                nc.scalar.activation(
                    ot[:, n0 : n0 + nw], pt[:, :nw], mybir.ActivationFunctionType.Copy
                )
                src = ot[:, n0 : n0 + nw]
            else:
                src = pt[:, :nw]
            # scale+bias on DVE once sc_rep is ready
            nc.vector.scalar_tensor_tensor(
                out=ot2[:, n0 : n0 + nw],
                in0=src,
                scalar=sc_rep[:],
                in1=b_rep[:, n0 : n0 + nw],
                op0=mybir.AluOpType.mult,
                op1=mybir.AluOpType.add,
            )
            cnt[0] += 1
            if cnt[0] == NT:
                nc.sync.dma_start(out3[:, mt], ot2[:])
                del ots[mt]

        # prologue: pair-major waves across the first NPRE tiles
        pre = [psum.tile([P, N_STEP], F32, name="pt") for _ in range(NPRE)]
        for kp in range(KP):
            for ti in range(NPRE):
                mt, n0, nw = tiles[ti]
                mm(pre[ti], mt, kp, False, n0, nw, kp == 0, False)
        for kp in range(LO_PAIRS):
            for ti in range(NPRE):
                mt, n0, nw = tiles[ti]
                mm(pre[ti], mt, kp, True, n0, nw, False, kp == LO_PAIRS - 1)
        for ti in range(NPRE):
            mt, n0, nw = tiles[ti]
            drain(pre[ti], mt, n0, nw)

        # steady state: tile-major
        for idx in range(NPRE, len(tiles)):
            mt, n0, nw = tiles[idx]
            pt = psum.tile([P, N_STEP], F32, name="pt")
            acc = 0
            for kp in range(KP):
                mm(pt, mt, kp, False, n0, nw, acc == 0, acc == n_acc - 1)
                acc += 1
            for kp in range(LO_PAIRS):
                mm(pt, mt, kp, True, n0, nw, acc == 0, acc == n_acc - 1)
                acc += 1
            drain(pt, mt, n0, nw)
    _split_multi_waits(nc)
    return nc


# ----------------------------------------------------------------------------
# Launch A (fallback, fold_scale=False): per-core sum of |w_shard| from bf16
# ----------------------------------------------------------------------------


def build_reduce_kernel() -> bass.Bass:
    nc = bass.Bass("TRN2", target_bir_lowering=False, debug=False)
    wt = nc.dram_tensor("wt", [DIN, DOUT_SH], BF16, kind="ExternalInput").ap()
    psum_out = nc.dram_tensor("psum_out", [1, 1], F32, kind="ExternalOutput").ap()
    wt3 = wt.rearrange("(ko p) n -> p ko n", p=P)

    KB = 4
    NCH = KO // KB

    with tile.TileContext(nc) as tc, ExitStack() as ctx:
        wpool = ctx.enter_context(tc.tile_pool(name="w", bufs=3))
        spool = ctx.enter_context(tc.tile_pool(name="s", bufs=1))
        sums = spool.tile([P, NCH], F32)
        for ch in range(NCH):
            wtile = wpool.tile([P, KB, DOUT_SH], BF16)
            nc.sync.dma_start(wtile[:], wt3[:, ch * KB : (ch + 1) * KB])
            nc.vector.tensor_reduce(
                sums[:, ch : ch + 1],
                wtile[:],
                axis=mybir.AxisListType.XY,
                op=mybir.AluOpType.add,
                apply_absolute_value=True,
            )
        tot = spool.tile([P, 1], F32)
        nc.vector.tensor_reduce(
            tot[:], sums[:], axis=mybir.AxisListType.X, op=mybir.AluOpType.add
        )
        ones = spool.tile([P, 1], F32)
        nc.vector.memset(ones[:], 1.0)
        pp = ctx.enter_context(tc.tile_pool(name="pp", bufs=1, space="PSUM"))
        acc = pp.tile([1, 1], F32)
        nc.tensor.matmul(acc[:], ones[:], tot[:], start=True, stop=True)
        tot2 = spool.tile([1, 1], F32)
        nc.vector.tensor_copy(out=tot2[:], in_=acc[:])
        nc.sync.dma_start(psum_out[:], tot2[:])
    _split_multi_waits(nc)
    return nc


# ----------------------------------------------------------------------------
# Host wrapper
# ----------------------------------------------------------------------------

_KERNEL_CACHE: dict = {}

FOLD_SCALE = os.environ.get("BITLINEAR_FOLD", "1") == "1"


def _get_main():
    key = ("main", FOLD_SCALE, LO_PAIRS)
    if key not in _KERNEL_CACHE:
        _KERNEL_CACHE[key] = build_main_kernel(fold_scale=FOLD_SCALE)
    return _KERNEL_CACHE[key]


def _get_reduce():
    if "A" not in _KERNEL_CACHE:
        _KERNEL_CACHE["A"] = build_reduce_kernel()
    return _KERNEL_CACHE["A"]


def _run_spmd(nc, in_maps, **kw):
    return run_bass_kernel_spmd(nc, in_maps, list(range(N_CORES)), **kw)


def _parallel(fn, n, threads=16):
    from concurrent.futures import ThreadPoolExecutor

    with ThreadPoolExecutor(threads) as ex:
        list(ex.map(fn, range(n)))


def prep_x(x: np.ndarray):
    """x [B,S,DIN] f32 -> (xt_hi [DIN,M] e4m3, xt_lo [KLO,M] e4m3)."""
    x2 = x.reshape(M, DIN)
    xt_hi = np.empty((DIN, M), NPF8)
    xt_lo = np.empty((KLO, M), NPF8)
    nblk = 32
    blk = DIN // nblk

    def run(i):
        s = slice(i * blk, (i + 1) * blk)
        col = np.ascontiguousarray(x2[:, s].T)  # [blk, M] f32
        hi = col.astype(NPF8)
        xt_hi[s] = hi
        lo0 = max(0, min(KLO, s.stop) - s.start)
        if lo0 > 0:
            res = col[:lo0] - hi[:lo0].astype(np.float32)
            xt_lo[s.start : s.start + lo0] = res.astype(NPF8)

    _parallel(run, nblk)
    return xt_hi, xt_lo


def prep_w(weight: np.ndarray):
    """weight [DOUT, DIN] f32 -> list of [DIN, DOUT_SH] bf16 shards."""
    shards = [np.empty((DIN, DOUT_SH), NPBF16) for _ in range(N_CORES)]

    def run(c):
        shards[c][:] = weight[c * DOUT_SH : (c + 1) * DOUT_SH].T.astype(NPBF16)

    _parallel(run, N_CORES, threads=N_CORES)
    return shards


def _execute(x, weight, bias, trace=False, tmpdirs=None):
    """Run all launches; returns (out, launches) where launches is a list of
    BassKernelResults (one per launch, in order)."""
    xt_hi, xt_lo = prep_x(x)
    wt_shards = prep_w(weight)
    bias_shards = [
        np.ascontiguousarray(bias[c * DOUT_SH : (c + 1) * DOUT_SH].reshape(1, -1))
        for c in range(N_CORES)
    ]
    kw = lambda i: (
        {"trace": True, "tmpdir": tmpdirs[i] if tmpdirs else None} if trace else {}
    )

    launches = []
    nc_main = _get_main()
    if FOLD_SCALE:
        in_maps = [
            {"xhi": xt_hi, "xlo": xt_lo, "wt": wt_shards[c], "bias": bias_shards[c]}
            for c in range(N_CORES)
        ]
        res = _run_spmd(nc_main, in_maps, **kw(0))
        launches.append(res)
    else:
        nc_a = _get_reduce()
        res_a = _run_spmd(nc_a, [{"wt": w} for w in wt_shards], **kw(0))
        launches.append(res_a)
        total = sum(float(res_a.results[c]["psum_out"][0, 0]) for c in range(N_CORES))
        scale_arr = np.full((1, 1), np.float32(total / (DOUT * DIN)), np.float32)
        in_maps = [
            {
                "xhi": xt_hi,
                "xlo": xt_lo,
                "wt": wt_shards[c],
                "bias": bias_shards[c],
                "scale": scale_arr,
            }
            for c in range(N_CORES)
        ]
        res = _run_spmd(nc_main, in_maps, **kw(1))
        launches.append(res)

    out = np.concatenate(
        [res.results[c]["out"] for c in range(N_CORES)], axis=1
    ).reshape(B, S, DOUT)
    return out, launches


def kernel(x: np.ndarray, weight: np.ndarray, bias: np.ndarray, **_ignored):
    x = np.asarray(x, dtype=np.float32)
    weight = np.asarray(weight, dtype=np.float32)
    bias = np.asarray(bias, dtype=np.float32)
    assert x.shape == (B, S, DIN) and weight.shape == (DOUT, DIN)
    out, _ = _execute(x, weight, bias)
    return out



# revision 2
# speedup vs baseline: 1.1124x; 1.1124x over previous
"""BitLinear kernel for Trainium2, tensor-parallel over 8 NeuronCores.

Reference computation:
    w_q = sign(weight) * mean(|weight|)      # weight [DOUT, DIN]
    out = x @ w_q.T + bias                   # x [B, S, DIN] -> out [B, S, DOUT]

Strategy (v3, fp8 DoubleRow + host-side sign/scale):
  - The PE runs fp8e4m3 matmuls with perf_mode=DoubleRow at 2x bf16
    throughput.  sign(w) is exact in fp8; the only quantization error
    is on x.
  - x is split hi/lo on the host: hi = e4m3(x) over the full K=4096,
    lo = e4m3(x - hi) over the first KLO rows of K.  Error budget:
    sqrt((1-f))*C with f = KLO/K and C ~ 0.02655 measured per-element
    e4m3 error; LO_PAIRS=8 -> f=0.5 -> l2 rel err 0.01877 (gate 2e-2).
  - v3: sign(w) is computed on the HOST and shipped as fp8 directly
    (5.6MB/core instead of 11.3MB bf16), and scale = mean|w| ships as
    a [1,1] f32 input.  This removes the on-device sign (ACT), the
    |w| reduction (DVE), the cross-core AllReduce of the scale, and
    the ACT copy-drains that existed only to decouple drains from the
    late-arriving scale.  All drains are direct PSUM -> STT on DVE.
  - Prologue interleaves hi/lo waves (hi0,lo0,hi1,lo1,...) across the
    8 PSUM banks so the PE consumes one NEW wq pair per two waves
    while the wq DMA stream delivers one pair per wave-time: the PE
    is DMA-gated only for the first ~1.5us.
  - Group-0 x is DMA'd per k-pair (24 small transfers) so the first
    stationary tile lands ~1.5us after launch; later groups use one
    bulk transfer per hi/lo stream.
"""

import os
import sys

for _p in ("/opt/trn_rl_repo",):
    if _p not in sys.path:
        sys.path.insert(0, _p)

from contextlib import ExitStack

import numpy as np
import ml_dtypes

import concourse.bass as bass
import concourse.tile as tile
from concourse import mybir
from concourse.bass_utils import run_bass_kernel_spmd

# ----------------------------------------------------------------------------
# Workaround for a walrus codegen limitation in this container: instructions
# can only encode ONE sync wait; insert single-wait NOPs for the rest.
# ----------------------------------------------------------------------------


def _mint_nop(nc, engine):
    inst = nc.engines[engine].nop(nofuse=True, hint="wsplit").ins
    bb = nc.cur_bb.bb
    lst = bb.instructions
    assert lst[-1].name == inst.name
    lst.pop()
    bb.instructions = lst
    return inst


def _split_multi_waits(nc):
    for fn in nc.m.functions:
        for bb in fn.blocks:
            insts = bb.instructions
            if not any(
                i.sync_info and i.sync_info.on_wait and len(i.sync_info.on_wait) > 1
                for i in insts
            ):
                continue
            new = []
            for inst in insts:
                si = inst.sync_info
                if si and si.on_wait and len(si.on_wait) > 1:
                    waits = list(si.on_wait)
                    for w in waits[:-1]:
                        nop = _mint_nop(nc, inst.engine)
                        nop.sync_info = mybir.SyncInfo(on_wait=[w], on_update=[])
                        new.append(nop)
                    si.on_wait = [waits[-1]]
                new.append(inst)
            bb.instructions = new


# ----------------------------------------------------------------------------
# Problem constants (hardcoded per contract)
# ----------------------------------------------------------------------------

B, S, DIN, DOUT = 2, 4096, 4096, 11008
N_CORES = 8
M = B * S
DOUT_SH = DOUT // N_CORES  # 1376
P = 128
KO = DIN // P  # 32 k-subtiles
KP = KO // 2  # 16 DoubleRow pairs
MT = M // P  # 64 row tiles
LO_PAIRS = int(os.environ.get("BITLINEAR_LO_PAIRS", "8"))
KLO = LO_PAIRS * 256
F32 = mybir.dt.float32
FP8 = mybir.dt.float8e4
NPF8 = ml_dtypes.float8_e4m3
DR = mybir.MatmulPerfMode.DoubleRow

X_W = 512
SUB = X_W // P
N_STEP = 512


def _n_slices(total: int, step: int):
    out = []
    o = 0
    while o < total:
        out.append((o, min(step, total - o)))
        o += step
    return out


# ----------------------------------------------------------------------------
# Main kernel
# ----------------------------------------------------------------------------


def build_main_kernel() -> bass.Bass:
    nc = bass.Bass("TRN2", target_bir_lowering=False, debug=False)
    xhi = nc.dram_tensor("xhi", [DIN, M], FP8, kind="ExternalInput").ap()
    xlo = nc.dram_tensor("xlo", [KLO, M], FP8, kind="ExternalInput").ap()
    ws = nc.dram_tensor("ws", [DIN, DOUT_SH], FP8, kind="ExternalInput").ap()
    bias = nc.dram_tensor("bias", [1, DOUT_SH], F32, kind="ExternalInput").ap()
    scale_in = nc.dram_tensor("scale", [1, 1], F32, kind="ExternalInput").ap()
    out = nc.dram_tensor("out", [M, DOUT_SH], F32, kind="ExternalOutput").ap()

    xhi4 = xhi.rearrange("(kp i p) m -> p kp i m", i=2, p=P)  # [128,16,2,M]
    xlo4 = xlo.rearrange("(kp i p) m -> p kp i m", i=2, p=P)  # [128,LO,2,M]
    ws4 = ws.rearrange("(kp i p) n -> p kp i n", i=2, p=P)  # [128,16,2,DOUT_SH]
    out3 = out.rearrange("(mt p) n -> p mt n", p=P)  # [128,64,DOUT_SH]

    nsl = _n_slices(DOUT_SH, N_STEP)
    NT = len(nsl)

    with tile.TileContext(nc) as tc, ExitStack() as ctx:
        const = ctx.enter_context(tc.tile_pool(name="const", bufs=1))
        xhp = ctx.enter_context(tc.tile_pool(name="xhp", bufs=3))
        xlp = ctx.enter_context(tc.tile_pool(name="xlp", bufs=3))
        outp2 = ctx.enter_context(tc.tile_pool(name="outp2", bufs=6))
        psum = ctx.enter_context(tc.tile_pool(name="psum", bufs=8, space="PSUM"))

        # --- wq: host-signed fp8, DMA'd directly; one transfer per pair on
        # the sync ring so the first pair lands ~1us in ---
        wq = [
            const.tile([P, 2, DOUT_SH], FP8, tag=f"wq{kp}", name=f"wq{kp}")
            for kp in range(KP)
        ]
        for kp in range(KP):
            nc.sync.dma_start(wq[kp][:], ws4[:, kp])

        # --- x group 0 on gpsimd, per-pair transfers interleaved hi/lo in
        # prologue consumption order ---
        xg = {}
        xh0 = xhp.tile([P, KP, 2, X_W], FP8, name="xh")
        xl0 = xlp.tile([P, LO_PAIRS, 2, X_W], FP8, name="xl")
        for kp in range(KP):
            nc.gpsimd.dma_start(
                xh0[:, kp : kp + 1], xhi4[:, kp : kp + 1, :, 0:X_W]
            )
            if kp < LO_PAIRS:
                nc.gpsimd.dma_start(
                    xl0[:, kp : kp + 1], xlo4[:, kp : kp + 1, :, 0:X_W]
                )
        xg[0] = (xh0, xl0)

        # --- scale + bias broadcasts (tiny; after group-0 x, before group 1;
        # first consumer is the first drain at ~40us) ---
        sc_rep = const.tile([P, 1], F32)
        nc.gpsimd.dma_start(sc_rep[0:1, :], scale_in[:])
        n = 1
        while n < P:
            nc.gpsimd.dma_start(sc_rep[n : 2 * n, :], sc_rep[0:n, :])
            n *= 2
        b_rep = const.tile([P, DOUT_SH], F32)
        nc.gpsimd.dma_start(b_rep[0:1, :], bias[:])
        n = 1
        while n < P:
            nc.gpsimd.dma_start(b_rep[n : 2 * n, :], b_rep[0:n, :])
            n *= 2

        # x group 1 prefetch (bulk)
        xh1 = xhp.tile([P, KP, 2, X_W], FP8, name="xh")
        nc.gpsimd.dma_start(xh1[:], xhi4[:, :, :, X_W : 2 * X_W])
        xl1 = xlp.tile([P, LO_PAIRS, 2, X_W], FP8, name="xl")
        nc.gpsimd.dma_start(xl1[:], xlo4[:, :, :, X_W : 2 * X_W])
        xg[1] = (xh1, xl1)

        # --- main loop over a flat tile list ---
        # tiles[idx] = (mt, n0, nw); mt = idx // NT.
        NPRE = 8
        n_acc = KP + LO_PAIRS
        tiles = [(mt, *nsl[i]) for mt in range(MT) for i in range(NT)]

        def get_x(mt):
            g = mt // SUB
            if g not in xg:
                xh = xhp.tile([P, KP, 2, X_W], FP8, name="xh")
                nc.gpsimd.dma_start(xh[:], xhi4[:, :, :, g * X_W : (g + 1) * X_W])
                xl = xlp.tile([P, LO_PAIRS, 2, X_W], FP8, name="xl")
                nc.gpsimd.dma_start(xl[:], xlo4[:, :, :, g * X_W : (g + 1) * X_W])
                xg[g] = (xh, xl)
            return xg[g], mt % SUB

        ots = {}

        def get_ot(mt):
            if mt not in ots:
                ots[mt] = (outp2.tile([P, DOUT_SH], F32, name="ot2"), [0])
            return ots[mt]

        def mm(pt, mt, kp, lo, n0, nw, start, stop):
            (xh, xl), s = get_x(mt)
            xs = xl if lo else xh
            nc.tensor.matmul(
                pt[:, :nw],
                xs[:, kp, :, s * P : (s + 1) * P],
                wq[kp][:, :, n0 : n0 + nw],
                start=start,
                stop=stop,
                perf_mode=DR,
            )

        def drain(pt, mt, n0, nw):
            ot2, cnt = get_ot(mt)
            nc.vector.scalar_tensor_tensor(
                out=ot2[:, n0 : n0 + nw],
                in0=pt[:, :nw],
                scalar=sc_rep[:],
                in1=b_rep[:, n0 : n0 + nw],
                op0=mybir.AluOpType.mult,
                op1=mybir.AluOpType.add,
            )
            cnt[0] += 1
            if cnt[0] == NT:
                nc.sync.dma_start(out3[:, mt], ot2[:])
                del ots[mt]

        # prologue: interleaved hi/lo pair-major waves across the first NPRE
        # tiles; each NEW wq pair feeds two waves (hi kp, lo kp) so the PE
        # tracks the wq/x DMA streams without stalling.
        waves = []
        for kp in range(KP):
            waves.append((kp, False))
            if kp < LO_PAIRS:
                waves.append((kp, True))
        pre = [psum.tile([P, N_STEP], F32, name="pt") for _ in range(NPRE)]
        for wi, (kp, lo) in enumerate(waves):
            for ti in range(NPRE):
                mt, n0, nw = tiles[ti]
                mm(pre[ti], mt, kp, lo, n0, nw, wi == 0, wi == len(waves) - 1)
        for ti in range(NPRE):
            mt, n0, nw = tiles[ti]
            drain(pre[ti], mt, n0, nw)

        # steady state: tile-major
        for idx in range(NPRE, len(tiles)):
            mt, n0, nw = tiles[idx]
            pt = psum.tile([P, N_STEP], F32, name="pt")
            acc = 0
            for kp in range(KP):
                mm(pt, mt, kp, False, n0, nw, acc == 0, acc == n_acc - 1)
                acc += 1
            for kp in range(LO_PAIRS):
                mm(pt, mt, kp, True, n0, nw, acc == 0, acc == n_acc - 1)
                acc += 1
            drain(pt, mt, n0, nw)
    _split_multi_waits(nc)
    return nc


# ----------------------------------------------------------------------------
# Host wrapper
# ----------------------------------------------------------------------------

_KERNEL_CACHE: dict = {}


def _get_main():
    key = ("main", LO_PAIRS)
    if key not in _KERNEL_CACHE:
        _KERNEL_CACHE[key] = build_main_kernel()
    return _KERNEL_CACHE[key]


def _run_spmd(nc, in_maps, **kw):
    return run_bass_kernel_spmd(nc, in_maps, list(range(N_CORES)), **kw)


def _parallel(fn, n, threads=16):
    from concurrent.futures import ThreadPoolExecutor

    with ThreadPoolExecutor(threads) as ex:
        list(ex.map(fn, range(n)))


def prep_x(x: np.ndarray):
    """x [B,S,DIN] f32 -> (xt_hi [DIN,M] e4m3, xt_lo [KLO,M] e4m3)."""
    x2 = x.reshape(M, DIN)
    xt_hi = np.empty((DIN, M), NPF8)
    xt_lo = np.empty((KLO, M), NPF8)
    nblk = 32
    blk = DIN // nblk

    def run(i):
        s = slice(i * blk, (i + 1) * blk)
        col = np.ascontiguousarray(x2[:, s].T)  # [blk, M] f32
        hi = col.astype(NPF8)
        xt_hi[s] = hi
        lo0 = max(0, min(KLO, s.stop) - s.start)
        if lo0 > 0:
            res = col[:lo0] - hi[:lo0].astype(np.float32)
            xt_lo[s.start : s.start + lo0] = res.astype(NPF8)

    _parallel(run, nblk)
    return xt_hi, xt_lo


def prep_w(weight: np.ndarray):
    """weight [DOUT, DIN] f32 -> list of [DIN, DOUT_SH] fp8 sign shards."""
    shards = [np.empty((DIN, DOUT_SH), NPF8) for _ in range(N_CORES)]

    def run(c):
        sh = weight[c * DOUT_SH : (c + 1) * DOUT_SH].T  # [DIN, DOUT_SH] view
        shards[c][:] = np.sign(sh).astype(NPF8)

    _parallel(run, N_CORES, threads=N_CORES)
    return shards


def _execute(x, weight, bias, trace=False, tmpdirs=None):
    """Run the launch; returns (out, launches) where launches is a list of
    BassKernelResults (one per launch, in order)."""
    xt_hi, xt_lo = prep_x(x)
    ws_shards = prep_w(weight)
    bias_shards = [
        np.ascontiguousarray(bias[c * DOUT_SH : (c + 1) * DOUT_SH].reshape(1, -1))
        for c in range(N_CORES)
    ]
    scale_arr = np.full((1, 1), np.float32(np.mean(np.abs(weight))), np.float32)
    kw = lambda i: (
        {"trace": True, "tmpdir": tmpdirs[i] if tmpdirs else None} if trace else {}
    )

    nc_main = _get_main()
    in_maps = [
        {
            "xhi": xt_hi,
            "xlo": xt_lo,
            "ws": ws_shards[c],
            "bias": bias_shards[c],
            "scale": scale_arr,
        }
        for c in range(N_CORES)
    ]
    res = _run_spmd(nc_main, in_maps, **kw(0))
    launches = [res]

    out = np.concatenate(
        [res.results[c]["out"] for c in range(N_CORES)], axis=1
    ).reshape(B, S, DOUT)
    return out, launches


def kernel(x: np.ndarray, weight: np.ndarray, bias: np.ndarray, **_ignored):
    x = np.asarray(x, dtype=np.float32)
    weight = np.asarray(weight, dtype=np.float32)
    bias = np.asarray(bias, dtype=np.float32)
    assert x.shape == (B, S, DIN) and weight.shape == (DOUT, DIN)
    out, _ = _execute(x, weight, bias)
    return out


# revision 6
# speedup vs baseline: 1.1598x; 1.0426x over previous
"""BitLinear kernel for Trainium2, tensor-parallel over 8 NeuronCores.

Reference computation:
    w_q = sign(weight) * mean(|weight|)      # weight [DOUT, DIN]
    out = x @ w_q.T + bias                   # x [B, S, DIN] -> out [B, S, DOUT]

Strategy (v3, fp8 DoubleRow + host-side sign/scale):
  - The PE runs fp8e4m3 matmuls with perf_mode=DoubleRow at 2x bf16
    throughput.  sign(w) is exact in fp8; the only quantization error
    is on x.
  - x is split hi/lo on the host: hi = e4m3(x) over the full K=4096,
    lo = e4m3(x - hi) over the first KLO rows of K.  Error budget:
    sqrt((1-f))*C with f = KLO/K and C ~ 0.02655 measured per-element
    e4m3 error; LO_PAIRS=8 -> f=0.5 -> l2 rel err 0.01877 (gate 2e-2).
  - v3: sign(w) is computed on the HOST and shipped as fp8 directly
    (5.6MB/core instead of 11.3MB bf16), and scale = mean|w| ships as
    a [1,1] f32 input.  This removes the on-device sign (ACT), the
    |w| reduction (DVE), the cross-core AllReduce of the scale, and
    the ACT copy-drains that existed only to decouple drains from the
    late-arriving scale.  All drains are direct PSUM -> STT on DVE.
  - Prologue interleaves hi/lo waves (hi0,lo0,hi1,lo1,...) across the
    8 PSUM banks so the PE consumes one NEW wq pair per two waves
    while the wq DMA stream delivers one pair per wave-time: the PE
    is DMA-gated only for the first ~1.5us.
  - Group-0 x is DMA'd per k-pair (24 small transfers) so the first
    stationary tile lands ~1.5us after launch; later groups use one
    bulk transfer per hi/lo stream.
"""

import os
import sys

for _p in ("/opt/trn_rl_repo",):
    if _p not in sys.path:
        sys.path.insert(0, _p)

from contextlib import ExitStack

import numpy as np
import ml_dtypes

import concourse.bass as bass
import concourse.tile as tile
from concourse import mybir
from concourse.bass_utils import run_bass_kernel_spmd

# ----------------------------------------------------------------------------
# Workaround for a walrus codegen limitation in this container: instructions
# can only encode ONE sync wait; insert single-wait NOPs for the rest.
# ----------------------------------------------------------------------------


def _mint_nop(nc, engine):
    inst = nc.engines[engine].nop(nofuse=True, hint="wsplit").ins
    bb = nc.cur_bb.bb
    lst = bb.instructions
    assert lst[-1].name == inst.name
    lst.pop()
    bb.instructions = lst
    return inst


def _split_multi_waits(nc):
    for fn in nc.m.functions:
        for bb in fn.blocks:
            insts = bb.instructions
            if not any(
                i.sync_info and i.sync_info.on_wait and len(i.sync_info.on_wait) > 1
                for i in insts
            ):
                continue
            new = []
            for inst in insts:
                si = inst.sync_info
                if si and si.on_wait and len(si.on_wait) > 1:
                    waits = list(si.on_wait)
                    for w in waits[:-1]:
                        nop = _mint_nop(nc, inst.engine)
                        nop.sync_info = mybir.SyncInfo(on_wait=[w], on_update=[])
                        new.append(nop)
                    si.on_wait = [waits[-1]]
                new.append(inst)
            bb.instructions = new


# ----------------------------------------------------------------------------
# Problem constants (hardcoded per contract)
# ----------------------------------------------------------------------------

B, S, DIN, DOUT = 2, 4096, 4096, 11008
N_CORES = 8
M = B * S
DOUT_SH = DOUT // N_CORES  # 1376
P = 128
KO = DIN // P  # 32 k-subtiles
KP = KO // 2  # 16 DoubleRow pairs
MT = M // P  # 64 row tiles
LO_PAIRS = int(os.environ.get("BITLINEAR_LO_PAIRS", "8"))
KLO = LO_PAIRS * 256
F32 = mybir.dt.float32
FP8 = mybir.dt.float8e4
NPF8 = ml_dtypes.float8_e4m3
DR = mybir.MatmulPerfMode.DoubleRow

X_W = 512
SUB = X_W // P
N_STEP = 512


def _n_slices(total: int, step: int):
    out = []
    o = 0
    while o < total:
        out.append((o, min(step, total - o)))
        o += step
    return out


# ----------------------------------------------------------------------------
# Main kernel
# ----------------------------------------------------------------------------


def build_main_kernel() -> bass.Bass:
    nc = bass.Bass("TRN2", target_bir_lowering=False, debug=False)
    xhi = nc.dram_tensor("xhi", [DIN, M], FP8, kind="ExternalInput").ap()
    xlo = nc.dram_tensor("xlo", [KLO, M], FP8, kind="ExternalInput").ap()
    ws = nc.dram_tensor("ws", [DIN, DOUT_SH], FP8, kind="ExternalInput").ap()
    # bias/scale arrive pre-replicated across the 128 partitions: a single
    # bulk DMA each.  (v3's on-device doubling chains serialized 14 dependent
    # DMAs on the busy gpsimd ring and did not finish until ~98us, blocking
    # the first drains and with them PSUM bank reuse for 44us.)
    bias = nc.dram_tensor("bias", [P, DOUT_SH], F32, kind="ExternalInput").ap()
    scale_in = nc.dram_tensor("scale", [P, 1], F32, kind="ExternalInput").ap()
    out = nc.dram_tensor("out", [M, DOUT_SH], F32, kind="ExternalOutput").ap()

    xhi4 = xhi.rearrange("(kp i p) m -> p kp i m", i=2, p=P)  # [128,16,2,M]
    xlo4 = xlo.rearrange("(kp i p) m -> p kp i m", i=2, p=P)  # [128,LO,2,M]
    ws4 = ws.rearrange("(kp i p) n -> p kp i n", i=2, p=P)  # [128,16,2,DOUT_SH]
    out3 = out.rearrange("(mt p) n -> p mt n", p=P)  # [128,64,DOUT_SH]

    nsl = _n_slices(DOUT_SH, N_STEP)
    NT = len(nsl)

    with tile.TileContext(nc) as tc, ExitStack() as ctx:
        const = ctx.enter_context(tc.tile_pool(name="const", bufs=1))
        xhp = ctx.enter_context(tc.tile_pool(name="xhp", bufs=3))
        xlp = ctx.enter_context(tc.tile_pool(name="xlp", bufs=3))
        outp2 = ctx.enter_context(tc.tile_pool(name="outp2", bufs=6))
        psum = ctx.enter_context(tc.tile_pool(name="psum", bufs=8, space="PSUM"))

        # --- wq: host-signed fp8, DMA'd directly; one transfer per pair on
        # the sync ring so the first pair lands ~1us in ---
        wq = [
            const.tile([P, 2, DOUT_SH], FP8, tag=f"wq{kp}", name=f"wq{kp}")
            for kp in range(KP)
        ]
        for kp in range(KP):
            nc.sync.dma_start(wq[kp][:], ws4[:, kp])

        # --- x group 0 on gpsimd, per-pair transfers interleaved hi/lo in
        # prologue consumption order ---
        xg = {}
        xh0 = xhp.tile([P, KP, 2, X_W], FP8, name="xh")
        xl0 = xlp.tile([P, LO_PAIRS, 2, X_W], FP8, name="xl")
        for kp in range(KP):
            nc.gpsimd.dma_start(
                xh0[:, kp : kp + 1], xhi4[:, kp : kp + 1, :, 0:X_W]
            )
            if kp < LO_PAIRS:
                nc.gpsimd.dma_start(
                    xl0[:, kp : kp + 1], xlo4[:, kp : kp + 1, :, 0:X_W]
                )
        xg[0] = (xh0, xl0)

        # --- scale + bias, pre-replicated on host: one DMA each (first
        # consumer is the first drain at ~40us) ---
        sc_rep = const.tile([P, 1], F32)
        nc.gpsimd.dma_start(sc_rep[:], scale_in[:])
        b_rep = const.tile([P, DOUT_SH], F32)
        nc.gpsimd.dma_start(b_rep[:], bias[:])

        # x group 1 prefetch (bulk)
        xh1 = xhp.tile([P, KP, 2, X_W], FP8, name="xh")
        nc.gpsimd.dma_start(xh1[:], xhi4[:, :, :, X_W : 2 * X_W])
        xl1 = xlp.tile([P, LO_PAIRS, 2, X_W], FP8, name="xl")
        nc.gpsimd.dma_start(xl1[:], xlo4[:, :, :, X_W : 2 * X_W])
        xg[1] = (xh1, xl1)

        # --- main loop over a flat tile list ---
        # tiles[idx] = (mt, n0, nw); mt = idx // NT.
        NPRE = 8
        n_acc = KP + LO_PAIRS
        tiles = [(mt, *nsl[i]) for mt in range(MT) for i in range(NT)]

        def get_x(mt):
            g = mt // SUB
            if g not in xg:
                xh = xhp.tile([P, KP, 2, X_W], FP8, name="xh")
                nc.gpsimd.dma_start(xh[:], xhi4[:, :, :, g * X_W : (g + 1) * X_W])
                xl = xlp.tile([P, LO_PAIRS, 2, X_W], FP8, name="xl")
                nc.gpsimd.dma_start(xl[:], xlo4[:, :, :, g * X_W : (g + 1) * X_W])
                xg[g] = (xh, xl)
            return xg[g], mt % SUB

        ots = {}

        def get_ot(mt):
            if mt not in ots:
                ots[mt] = (outp2.tile([P, DOUT_SH], F32, name="ot2"), [0])
            return ots[mt]

        def mm(pt, mt, kp, lo, n0, nw, start, stop):
            (xh, xl), s = get_x(mt)
            xs = xl if lo else xh
            nc.tensor.matmul(
                pt[:, :nw],
                xs[:, kp, :, s * P : (s + 1) * P],
                wq[kp][:, :, n0 : n0 + nw],
                start=start,
                stop=stop,
                perf_mode=DR,
            )

        def drain(pt, mt, n0, nw):
            ot2, cnt = get_ot(mt)
            nc.vector.scalar_tensor_tensor(
                out=ot2[:, n0 : n0 + nw],
                in0=pt[:, :nw],
                scalar=sc_rep[:],
                in1=b_rep[:, n0 : n0 + nw],
                op0=mybir.AluOpType.mult,
                op1=mybir.AluOpType.add,
            )
            cnt[0] += 1
            if mt == MT - 1:
                # tail trim: ship each chunk of the final row tile as soon as
                # its drain lands instead of waiting for the whole tile
                nc.sync.dma_start(out3[:, mt, n0 : n0 + nw], ot2[:, n0 : n0 + nw])
                if cnt[0] == NT:
                    del ots[mt]
            elif cnt[0] == NT:
                nc.sync.dma_start(out3[:, mt], ot2[:])
                del ots[mt]

        # prologue: interleaved hi/lo pair-major waves across the first NPRE
        # tiles; each NEW wq pair feeds two waves (hi kp, lo kp) so the PE
        # tracks the wq/x DMA streams without stalling.
        waves = []
        for kp in range(KP):
            waves.append((kp, False))
            if kp < LO_PAIRS:
                waves.append((kp, True))
        pre = [psum.tile([P, N_STEP], F32, name="pt") for _ in range(NPRE)]
        for wi, (kp, lo) in enumerate(waves):
            for ti in range(NPRE):
                mt, n0, nw = tiles[ti]
                mm(pre[ti], mt, kp, lo, n0, nw, wi == 0, wi == len(waves) - 1)
        for ti in range(NPRE):
            mt, n0, nw = tiles[ti]
            drain(pre[ti], mt, n0, nw)

        # steady state: tile-major
        for idx in range(NPRE, len(tiles)):
            mt, n0, nw = tiles[idx]
            pt = psum.tile([P, N_STEP], F32, name="pt")
            acc = 0
            for kp in range(KP):
                mm(pt, mt, kp, False, n0, nw, acc == 0, acc == n_acc - 1)
                acc += 1
            for kp in range(LO_PAIRS):
                mm(pt, mt, kp, True, n0, nw, acc == 0, acc == n_acc - 1)
                acc += 1
            drain(pt, mt, n0, nw)
    _split_multi_waits(nc)
    return nc


# ----------------------------------------------------------------------------
# Host wrapper
# ----------------------------------------------------------------------------

_KERNEL_CACHE: dict = {}


def _get_main():
    key = ("main", LO_PAIRS)
    if key not in _KERNEL_CACHE:
        _KERNEL_CACHE[key] = build_main_kernel()
    return _KERNEL_CACHE[key]


def _run_spmd(nc, in_maps, **kw):
    return run_bass_kernel_spmd(nc, in_maps, list(range(N_CORES)), **kw)


def _parallel(fn, n, threads=16):
    from concurrent.futures import ThreadPoolExecutor

    with ThreadPoolExecutor(threads) as ex:
        list(ex.map(fn, range(n)))


def prep_x(x: np.ndarray):
    """x [B,S,DIN] f32 -> (xt_hi [DIN,M] e4m3, xt_lo [KLO,M] e4m3)."""
    x2 = x.reshape(M, DIN)
    xt_hi = np.empty((DIN, M), NPF8)
    xt_lo = np.empty((KLO, M), NPF8)
    nblk = 32
    blk = DIN // nblk

    def run(i):
        s = slice(i * blk, (i + 1) * blk)
        col = np.ascontiguousarray(x2[:, s].T)  # [blk, M] f32
        hi = col.astype(NPF8)
        xt_hi[s] = hi
        lo0 = max(0, min(KLO, s.stop) - s.start)
        if lo0 > 0:
            res = col[:lo0] - hi[:lo0].astype(np.float32)
            xt_lo[s.start : s.start + lo0] = res.astype(NPF8)

    _parallel(run, nblk)
    return xt_hi, xt_lo


def prep_w(weight: np.ndarray):
    """weight [DOUT, DIN] f32 -> list of [DIN, DOUT_SH] fp8 sign shards."""
    shards = [np.empty((DIN, DOUT_SH), NPF8) for _ in range(N_CORES)]

    def run(c):
        sh = weight[c * DOUT_SH : (c + 1) * DOUT_SH].T  # [DIN, DOUT_SH] view
        shards[c][:] = np.sign(sh).astype(NPF8)

    _parallel(run, N_CORES, threads=N_CORES)
    return shards


def _execute(x, weight, bias, trace=False, tmpdirs=None):
    """Run the launch; returns (out, launches) where launches is a list of
    BassKernelResults (one per launch, in order)."""
    xt_hi, xt_lo = prep_x(x)
    ws_shards = prep_w(weight)
    bias_shards = [
        np.ascontiguousarray(
            np.broadcast_to(
                bias[c * DOUT_SH : (c + 1) * DOUT_SH].reshape(1, -1), (P, DOUT_SH)
            )
        )
        for c in range(N_CORES)
    ]
    scale_arr = np.full((P, 1), np.float32(np.mean(np.abs(weight))), np.float32)
    kw = lambda i: (
        {"trace": True, "tmpdir": tmpdirs[i] if tmpdirs else None} if trace else {}
    )

    nc_main = _get_main()
    in_maps = [
        {
            "xhi": xt_hi,
            "xlo": xt_lo,
            "ws": ws_shards[c],
            "bias": bias_shards[c],
            "scale": scale_arr,
        }
        for c in range(N_CORES)
    ]
    res = _run_spmd(nc_main, in_maps, **kw(0))
    launches = [res]

    out = np.concatenate(
        [res.results[c]["out"] for c in range(N_CORES)], axis=1
    ).reshape(B, S, DOUT)
    return out, launches


def kernel(x: np.ndarray, weight: np.ndarray, bias: np.ndarray, **_ignored):
    x = np.asarray(x, dtype=np.float32)
    weight = np.asarray(weight, dtype=np.float32)
    bias = np.asarray(bias, dtype=np.float32)
    assert x.shape == (B, S, DIN) and weight.shape == (DOUT, DIN)
    out, _ = _execute(x, weight, bias)
    return out


# revision 7
# speedup vs baseline: 1.2070x; 1.0407x over previous
"""BitLinear kernel for Trainium2, tensor-parallel over 8 NeuronCores.

Reference computation:
    w_q = sign(weight) * mean(|weight|)      # weight [DOUT, DIN]
    out = x @ w_q.T + bias                   # x [B, S, DIN] -> out [B, S, DOUT]

Strategy (v3, fp8 DoubleRow + host-side sign/scale):
  - The PE runs fp8e4m3 matmuls with perf_mode=DoubleRow at 2x bf16
    throughput.  sign(w) is exact in fp8; the only quantization error
    is on x.
  - x is split hi/lo on the host: hi = e4m3(x) over the full K=4096,
    lo = e4m3(x - hi) over the first KLO rows of K.  Error budget:
    sqrt((1-f))*C with f = KLO/K and C ~ 0.02655 measured per-element
    e4m3 error; LO_PAIRS=8 -> f=0.5 -> l2 rel err 0.01877 (gate 2e-2).
  - v3: sign(w) is computed on the HOST and shipped as fp8 directly
    (5.6MB/core instead of 11.3MB bf16), and scale = mean|w| ships as
    a [1,1] f32 input.  This removes the on-device sign (ACT), the
    |w| reduction (DVE), the cross-core AllReduce of the scale, and
    the ACT copy-drains that existed only to decouple drains from the
    late-arriving scale.  All drains are direct PSUM -> STT on DVE.
  - Prologue interleaves hi/lo waves (hi0,lo0,hi1,lo1,...) across the
    8 PSUM banks so the PE consumes one NEW wq pair per two waves
    while the wq DMA stream delivers one pair per wave-time: the PE
    is DMA-gated only for the first ~1.5us.
  - Group-0 x is DMA'd per k-pair (24 small transfers) so the first
    stationary tile lands ~1.5us after launch; later groups use one
    bulk transfer per hi/lo stream.
"""

import os
import sys

for _p in ("/opt/trn_rl_repo",):
    if _p not in sys.path:
        sys.path.insert(0, _p)

from contextlib import ExitStack

import numpy as np
import ml_dtypes

import concourse.bass as bass
import concourse.tile as tile
from concourse import mybir
from concourse.bass_utils import run_bass_kernel_spmd

# ----------------------------------------------------------------------------
# Workaround for a walrus codegen limitation in this container: instructions
# can only encode ONE sync wait; insert single-wait NOPs for the rest.
# ----------------------------------------------------------------------------


def _mint_nop(nc, engine):
    inst = nc.engines[engine].nop(nofuse=True, hint="wsplit").ins
    bb = nc.cur_bb.bb
    lst = bb.instructions
    assert lst[-1].name == inst.name
    lst.pop()
    bb.instructions = lst
    return inst


def _split_multi_waits(nc):
    for fn in nc.m.functions:
        for bb in fn.blocks:
            insts = bb.instructions
            if not any(
                i.sync_info and i.sync_info.on_wait and len(i.sync_info.on_wait) > 1
                for i in insts
            ):
                continue
            new = []
            for inst in insts:
                si = inst.sync_info
                if si and si.on_wait and len(si.on_wait) > 1:
                    waits = list(si.on_wait)
                    for w in waits[:-1]:
                        nop = _mint_nop(nc, inst.engine)
                        nop.sync_info = mybir.SyncInfo(on_wait=[w], on_update=[])
                        new.append(nop)
                    si.on_wait = [waits[-1]]
                new.append(inst)
            bb.instructions = new


# ----------------------------------------------------------------------------
# Problem constants (hardcoded per contract)
# ----------------------------------------------------------------------------

B, S, DIN, DOUT = 2, 4096, 4096, 11008
N_CORES = 8
M = B * S
DOUT_SH = DOUT // N_CORES  # 1376
P = 128
KO = DIN // P  # 32 k-subtiles
KP = KO // 2  # 16 DoubleRow pairs
MT = M // P  # 64 row tiles
LO_PAIRS = int(os.environ.get("BITLINEAR_LO_PAIRS", "7"))
KLO = LO_PAIRS * 256
F32 = mybir.dt.float32
FP8 = mybir.dt.float8e4
NPF8 = ml_dtypes.float8_e4m3
DR = mybir.MatmulPerfMode.DoubleRow

X_W = 512
SUB = X_W // P
N_STEP = 512


def _n_slices(total: int, step: int):
    out = []
    o = 0
    while o < total:
        out.append((o, min(step, total - o)))
        o += step
    return out


# ----------------------------------------------------------------------------
# Main kernel
# ----------------------------------------------------------------------------


def build_main_kernel() -> bass.Bass:
    nc = bass.Bass("TRN2", target_bir_lowering=False, debug=False)
    xhi = nc.dram_tensor("xhi", [DIN, M], FP8, kind="ExternalInput").ap()
    xlo = nc.dram_tensor("xlo", [KLO, M], FP8, kind="ExternalInput").ap()
    ws = nc.dram_tensor("ws", [DIN, DOUT_SH], FP8, kind="ExternalInput").ap()
    # bias/scale arrive pre-replicated across the 128 partitions: a single
    # bulk DMA each.  (v3's on-device doubling chains serialized 14 dependent
    # DMAs on the busy gpsimd ring and did not finish until ~98us, blocking
    # the first drains and with them PSUM bank reuse for 44us.)
    bias = nc.dram_tensor("bias", [P, DOUT_SH], F32, kind="ExternalInput").ap()
    scale_in = nc.dram_tensor("scale", [P, 1], F32, kind="ExternalInput").ap()
    out = nc.dram_tensor("out", [M, DOUT_SH], F32, kind="ExternalOutput").ap()

    xhi4 = xhi.rearrange("(kp i p) m -> p kp i m", i=2, p=P)  # [128,16,2,M]
    xlo4 = xlo.rearrange("(kp i p) m -> p kp i m", i=2, p=P)  # [128,LO,2,M]
    ws4 = ws.rearrange("(kp i p) n -> p kp i n", i=2, p=P)  # [128,16,2,DOUT_SH]
    out3 = out.rearrange("(mt p) n -> p mt n", p=P)  # [128,64,DOUT_SH]

    nsl = _n_slices(DOUT_SH, N_STEP)
    NT = len(nsl)

    with tile.TileContext(nc) as tc, ExitStack() as ctx:
        const = ctx.enter_context(tc.tile_pool(name="const", bufs=1))
        xhp = ctx.enter_context(tc.tile_pool(name="xhp", bufs=3))
        xlp = ctx.enter_context(tc.tile_pool(name="xlp", bufs=3))
        outp2 = ctx.enter_context(tc.tile_pool(name="outp2", bufs=6))
        psum = ctx.enter_context(tc.tile_pool(name="psum", bufs=8, space="PSUM"))

        # --- wq: host-signed fp8, DMA'd directly; one transfer per pair on
        # the sync ring so the first pair lands ~1us in ---
        wq = [
            const.tile([P, 2, DOUT_SH], FP8, tag=f"wq{kp}", name=f"wq{kp}")
            for kp in range(KP)
        ]
        for kp in range(KP):
            nc.sync.dma_start(wq[kp][:], ws4[:, kp])

        # --- x group 0 on gpsimd, per-pair transfers interleaved hi/lo in
        # prologue consumption order ---
        xg = {}
        xh0 = xhp.tile([P, KP, 2, X_W], FP8, name="xh")
        xl0 = xlp.tile([P, LO_PAIRS, 2, X_W], FP8, name="xl")
        for kp in range(KP):
            nc.gpsimd.dma_start(
                xh0[:, kp : kp + 1], xhi4[:, kp : kp + 1, :, 0:X_W]
            )
            if kp < LO_PAIRS:
                nc.gpsimd.dma_start(
                    xl0[:, kp : kp + 1], xlo4[:, kp : kp + 1, :, 0:X_W]
                )
        xg[0] = (xh0, xl0)

        # --- scale + bias, pre-replicated on host: one DMA each (first
        # consumer is the first drain at ~40us) ---
        sc_rep = const.tile([P, 1], F32)
        nc.gpsimd.dma_start(sc_rep[:], scale_in[:])
        b_rep = const.tile([P, DOUT_SH], F32)
        nc.gpsimd.dma_start(b_rep[:], bias[:])

        # x group 1 prefetch (bulk)
        xh1 = xhp.tile([P, KP, 2, X_W], FP8, name="xh")
        nc.gpsimd.dma_start(xh1[:], xhi4[:, :, :, X_W : 2 * X_W])
        xl1 = xlp.tile([P, LO_PAIRS, 2, X_W], FP8, name="xl")
        nc.gpsimd.dma_start(xl1[:], xlo4[:, :, :, X_W : 2 * X_W])
        xg[1] = (xh1, xl1)

        # --- main loop over a flat tile list ---
        # tiles[idx] = (mt, n0, nw); mt = idx // NT.
        NPRE = 8
        n_acc = KP + LO_PAIRS
        tiles = [(mt, *nsl[i]) for mt in range(MT) for i in range(NT)]

        def get_x(mt):
            g = mt // SUB
            if g not in xg:
                xh = xhp.tile([P, KP, 2, X_W], FP8, name="xh")
                nc.gpsimd.dma_start(xh[:], xhi4[:, :, :, g * X_W : (g + 1) * X_W])
                xl = xlp.tile([P, LO_PAIRS, 2, X_W], FP8, name="xl")
                nc.gpsimd.dma_start(xl[:], xlo4[:, :, :, g * X_W : (g + 1) * X_W])
                xg[g] = (xh, xl)
            return xg[g], mt % SUB

        ots = {}

        def get_ot(mt):
            if mt not in ots:
                ots[mt] = (outp2.tile([P, DOUT_SH], F32, name="ot2"), [0])
            return ots[mt]

        def mm(pt, mt, kp, lo, n0, nw, start, stop):
            (xh, xl), s = get_x(mt)
            xs = xl if lo else xh
            nc.tensor.matmul(
                pt[:, :nw],
                xs[:, kp, :, s * P : (s + 1) * P],
                wq[kp][:, :, n0 : n0 + nw],
                start=start,
                stop=stop,
                perf_mode=DR,
            )

        def drain(pt, mt, n0, nw):
            ot2, cnt = get_ot(mt)
            nc.vector.scalar_tensor_tensor(
                out=ot2[:, n0 : n0 + nw],
                in0=pt[:, :nw],
                scalar=sc_rep[:],
                in1=b_rep[:, n0 : n0 + nw],
                op0=mybir.AluOpType.mult,
                op1=mybir.AluOpType.add,
            )
            cnt[0] += 1
            if mt == MT - 1:
                # tail trim: ship each chunk of the final row tile as soon as
                # its drain lands instead of waiting for the whole tile
                nc.sync.dma_start(out3[:, mt, n0 : n0 + nw], ot2[:, n0 : n0 + nw])
                if cnt[0] == NT:
                    del ots[mt]
            elif cnt[0] == NT:
                nc.sync.dma_start(out3[:, mt], ot2[:])
                del ots[mt]

        # prologue: interleaved hi/lo pair-major waves across the first NPRE
        # tiles; each NEW wq pair feeds two waves (hi kp, lo kp) so the PE
        # tracks the wq/x DMA streams without stalling.
        waves = []
        for kp in range(KP):
            waves.append((kp, False))
            if kp < LO_PAIRS:
                waves.append((kp, True))
        pre = [psum.tile([P, N_STEP], F32, name="pt") for _ in range(NPRE)]
        for wi, (kp, lo) in enumerate(waves):
            for ti in range(NPRE):
                mt, n0, nw = tiles[ti]
                mm(pre[ti], mt, kp, lo, n0, nw, wi == 0, wi == len(waves) - 1)
        for ti in range(NPRE):
            mt, n0, nw = tiles[ti]
            drain(pre[ti], mt, n0, nw)

        # steady state: tile-major
        for idx in range(NPRE, len(tiles)):
            mt, n0, nw = tiles[idx]
            pt = psum.tile([P, N_STEP], F32, name="pt")
            acc = 0
            for kp in range(KP):
                mm(pt, mt, kp, False, n0, nw, acc == 0, acc == n_acc - 1)
                acc += 1
            for kp in range(LO_PAIRS):
                mm(pt, mt, kp, True, n0, nw, acc == 0, acc == n_acc - 1)
                acc += 1
            drain(pt, mt, n0, nw)
    _split_multi_waits(nc)
    return nc


# ----------------------------------------------------------------------------
# Host wrapper
# ----------------------------------------------------------------------------

_KERNEL_CACHE: dict = {}


def _get_main():
    key = ("main", LO_PAIRS)
    if key not in _KERNEL_CACHE:
        _KERNEL_CACHE[key] = build_main_kernel()
    return _KERNEL_CACHE[key]


def _run_spmd(nc, in_maps, **kw):
    return run_bass_kernel_spmd(nc, in_maps, list(range(N_CORES)), **kw)


def _parallel(fn, n, threads=16):
    from concurrent.futures import ThreadPoolExecutor

    with ThreadPoolExecutor(threads) as ex:
        list(ex.map(fn, range(n)))


def prep_x(x: np.ndarray):
    """x [B,S,DIN] f32 -> (xt_hi [DIN,M] e4m3, xt_lo [KLO,M] e4m3)."""
    x2 = x.reshape(M, DIN)
    xt_hi = np.empty((DIN, M), NPF8)
    xt_lo = np.empty((KLO, M), NPF8)
    nblk = 32
    blk = DIN // nblk

    def run(i):
        s = slice(i * blk, (i + 1) * blk)
        col = np.ascontiguousarray(x2[:, s].T)  # [blk, M] f32
        hi = col.astype(NPF8)
        xt_hi[s] = hi
        lo0 = max(0, min(KLO, s.stop) - s.start)
        if lo0 > 0:
            res = col[:lo0] - hi[:lo0].astype(np.float32)
            xt_lo[s.start : s.start + lo0] = res.astype(NPF8)

    _parallel(run, nblk)
    return xt_hi, xt_lo


def prep_w(weight: np.ndarray):
    """weight [DOUT, DIN] f32 -> list of [DIN, DOUT_SH] fp8 sign shards."""
    shards = [np.empty((DIN, DOUT_SH), NPF8) for _ in range(N_CORES)]

    def run(c):
        sh = weight[c * DOUT_SH : (c + 1) * DOUT_SH].T  # [DIN, DOUT_SH] view
        shards[c][:] = np.sign(sh).astype(NPF8)

    _parallel(run, N_CORES, threads=N_CORES)
    return shards


def _execute(x, weight, bias, trace=False, tmpdirs=None):
    """Run the launch; returns (out, launches) where launches is a list of
    BassKernelResults (one per launch, in order)."""
    xt_hi, xt_lo = prep_x(x)
    ws_shards = prep_w(weight)
    bias_shards = [
        np.ascontiguousarray(
            np.broadcast_to(
                bias[c * DOUT_SH : (c + 1) * DOUT_SH].reshape(1, -1), (P, DOUT_SH)
            )
        )
        for c in range(N_CORES)
    ]
    scale_arr = np.full((P, 1), np.float32(np.mean(np.abs(weight))), np.float32)
    kw = lambda i: (
        {"trace": True, "tmpdir": tmpdirs[i] if tmpdirs else None} if trace else {}
    )

    nc_main = _get_main()
    in_maps = [
        {
            "xhi": xt_hi,
            "xlo": xt_lo,
            "ws": ws_shards[c],
            "bias": bias_shards[c],
            "scale": scale_arr,
        }
        for c in range(N_CORES)
    ]
    res = _run_spmd(nc_main, in_maps, **kw(0))
    launches = [res]

    out = np.concatenate(
        [res.results[c]["out"] for c in range(N_CORES)], axis=1
    ).reshape(B, S, DOUT)
    return out, launches


def kernel(x: np.ndarray, weight: np.ndarray, bias: np.ndarray, **_ignored):
    x = np.asarray(x, dtype=np.float32)
    weight = np.asarray(weight, dtype=np.float32)
    bias = np.asarray(bias, dtype=np.float32)
    assert x.shape == (B, S, DIN) and weight.shape == (DOUT, DIN)
    out, _ = _execute(x, weight, bias)
    return out
